# revision 7
# baseline (speedup 1.0000x reference)
"""InversePixelShuffle (pixel_unshuffle, k=2) Trainium2 Bass kernel.

Full input x: (8, 32, 512, 512) f32 -> output (8, 128, 256, 256) f32
  out[b, c*4 + iy*2 + ix, ho, wo] = x[b, c, 2*ho+iy, 2*wo+ix]

Sharding: batch dim across 8 NeuronCores (1 sample per core, no comms).

Per-core dataflow (raw Bass, manual semaphores):
  SBUF partition p holds output-row pairs (ho = 2p, 2p+1), so every DMA
  moves >=1KB contiguous HBM chunks. Per iteration (channel-group cg of 8,
  iy, r): SP issues a 2 MiB in-DMA (x rows 4p+2r+iy, 8 channels), DVE
  deinterleaves even columns, ACT odd columns, SP issues two 1 MiB
  out-DMAs. Software-pipelined with D in-buffers / E out-buffers.

All DMAs issue from the SP HWDGE ring (FIFO completion), so cumulative
semaphore thresholds are sound. Every DMA carries 0 inline waits and
every engine op 0 inline waits (walrus limits: DMA<=1, engine<=2);
ordering is done with standalone wait_ge instructions per engine queue.
"""

import sys

for p in ("/opt/trn_rl_repo",):
    if p not in sys.path:
        sys.path.insert(0, p)

import numpy as np

import concourse.mybir as mybir
from concourse import bass
from concourse.bass import Bass
from concourse.bass_utils import run_bass_kernel_spmd

N_CORES = 8
C, H, W = 32, 512, 512
HO, WO = H // 2, W // 2
CG = 8                      # channels per iteration
N_ITER = (C // CG) * 2 * 2  # cg x iy x r = 16
D = 4                       # in-tile pipeline depth
E = 4                       # out-tile pipeline depth

_cache = {}


def _build():
    if "nc" in _cache:
        return _cache["nc"]
    nc = Bass()
    x = nc.declare_dram_parameter("x", [C, H, W], mybir.dt.float32, isOutput=False)
    y = nc.declare_dram_parameter(
        "y", [4 * C, HO, WO], mybir.dt.float32, isOutput=True
    )

    # x_r[p, c, r, i, w] = x[c, 4p + 2r + i, w]
    x_r = x[:].rearrange("c (p r i) w -> p c r i w", p=128, r=2, i=2)
    # y_r[k, r, p, c, w] = y[4c + k, 2p + r, w]
    y_r = y[:].rearrange("(c k) (p r) w -> k r p c w", k=4, p=128, r=2)

    def iter_params(i):
        cg, rest = divmod(i, 4)
        iy, r = divmod(rest, 2)
        return cg * CG, iy, r

    with (
        nc.Block() as block,
        nc.semaphore("s_in") as s_in,
        nc.semaphore("s_ev") as s_ev,
        nc.semaphore("s_od") as s_od,
        nc.semaphore("s_out") as s_out,
    ):
        its, evs, ods = [], [], []
        for s in range(D):
            its.append(nc.sbuf_tensor(f"it{s}", [128, CG * W], mybir.dt.float32))
        for s in range(E):
            evs.append(nc.sbuf_tensor(f"ev{s}", [128, CG * WO], mybir.dt.float32))
            ods.append(nc.sbuf_tensor(f"od{s}", [128, CG * WO], mybir.dt.float32))
        import contextlib

        stack = contextlib.ExitStack()
        its = [stack.enter_context(t) for t in its]
        evs = [stack.enter_context(t) for t in evs]
        ods = [stack.enter_context(t) for t in ods]

        def it_v(s):
            return its[s][:].rearrange("p (c w) -> p c w", c=CG)

        def ev_v(s):
            return evs[s][:].rearrange("p (c w) -> p c w", c=CG)

        def od_v(s):
            return ods[s][:].rearrange("p (c w) -> p c w", c=CG)

        def out_ev(sync, i):
            c0, iy, r = iter_params(i)
            sync.dma_start(
                out=y_r[2 * iy + 0, r, :, c0 : c0 + CG, :], in_=ev_v(i % E)
            ).then_inc(s_out, 16)

        def out_od(sync, i):
            c0, iy, r = iter_params(i)
            sync.dma_start(
                out=y_r[2 * iy + 1, r, :, c0 : c0 + CG, :], in_=od_v(i % E)
            ).then_inc(s_out, 16)

        @block.sync
        def _(sync: bass.BassEngine):
            for i in range(N_ITER):
                if i >= D:
                    sync.wait_ge(s_ev, i - D + 1)
                    sync.wait_ge(s_od, i - D + 1)
                c0, iy, r = iter_params(i)
                sync.dma_start(
                    out=it_v(i % D), in_=x_r[:, c0 : c0 + CG, r, iy, :]
                ).then_inc(s_in, 16)
                if i >= 1:
                    sync.wait_ge(s_ev, i)
                    sync.wait_ge(s_od, i)
                    out_ev(sync, i - 1)
                    out_od(sync, i - 1)
            sync.wait_ge(s_ev, N_ITER)
            sync.wait_ge(s_od, N_ITER)
            out_ev(sync, N_ITER - 1)
            out_od(sync, N_ITER - 1)
            sync.wait_ge(s_out, 32 * N_ITER)

        @block.vector
        def _(vector: bass.BassEngine):
            for i in range(N_ITER):
                vector.wait_ge(s_in, 16 * (i + 1))
                if i >= E:
                    vector.wait_ge(s_out, 32 * (i - E + 1))
                vector.tensor_copy(
                    out=ev_v(i % E), in_=it_v(i % D)[:, :, 0::2]
                ).then_inc(s_ev, 1)

        @block.scalar
        def _(scalar: bass.BassEngine):
            for i in range(N_ITER):
                scalar.wait_ge(s_in, 16 * (i + 1))
                if i >= E:
                    scalar.wait_ge(s_out, 32 * (i - E + 1))
                scalar.copy(
                    out=od_v(i % E), in_=it_v(i % D)[:, :, 1::2]
                ).then_inc(s_od, 1)

        stack.close()

    if not nc.is_finalized():
        nc.finalize()
    _cache["nc"] = nc
    return nc


def run(x: np.ndarray, trace: bool = False):
    nc = _build()
    in_maps = [{"x": np.ascontiguousarray(x[b])} for b in range(N_CORES)]
    res = run_bass_kernel_spmd(
        nc, in_maps, core_ids=list(range(N_CORES)), trace=trace
    )
    out = np.stack([res.results[b]["y"] for b in range(N_CORES)], axis=0)
    return out, res


def kernel(x: np.ndarray) -> np.ndarray:
    out, _ = run(x, trace=False)
    return out
